# revision 1
# baseline (speedup 1.0000x reference)
"""BiLevelRoutingAttention (spiking) Trainium2 kernel.

Sharding: one (t, b) pair per core (T=4 x B=2 = 8 cores). All windows of a
(t,b) live on one core, so the routed-KV gather is local. The only cross-core
data is the routing region mean (over t and s), realized as a tiny [128,64]
AllReduce among the 4 cores sharing each b (replica groups {0..3}, {4..7};
core_id = b*4 + t).

Math structure (all on device, per core):
  qkv = xw @ w_qkv + b       -> spike = (qkv >= 2 - b) in {0,1}   (LIF lif())
  per-window Gram G_v = k_v^T [v_v | 1]  ([128 kcol, 129] per chunk; the ones
    column folds ksum into the Gram)
  routing: scores = Xsum Xsum^T (monotonic in the reference scores), top-4 per
    row as a {0,1} selection matrix (vector.max + match_replace), transposed
    via PE so sel^T can be the stationary operand
  combine: kv_r rows = sel^T.T @ G-rows  (Gram re-oriented window-major via a
    DRAM round trip)
  attention: out = kv_r_blockdiag.T @ q_T per (window, chunk); den via a
    column-replicated ksum lhsT; divide; project with w_proj; add b_proj.
"""

import numpy as np
import ml_dtypes

T, B, Lt, Lh, Lw, C = 4, 2, 8, 32, 32, 256
WT, WH, WW = 2, 4, 4
LT, LH, LW = Lt // WT, Lh // WH, Lw // WW  # 4, 8, 8
W = WT * WH * WW        # 32 windows
S = LT * LH * LW        # 256 tokens per window
NTOK = W * S            # 8192
H, D = 8, 32
TOPK = 4
NCORES = 8
GROUPS = [[0, 1, 2, 3], [4, 5, 6, 7]]
ROW = 2 * 128 * 129     # 33024 flat gram row length
CCH = 344               # combine N-chunk (96 chunks; 24 per quarter-row load)
BF16 = ml_dtypes.bfloat16

_CACHE = {}


def build_kernel():
    from concourse import bacc
    import concourse.mybir as mybir
    import concourse.tile as tile
    from concourse.tile_rust import add_dep_helper
    from concourse.masks import make_identity

    bf = mybir.dt.bfloat16
    f32 = mybir.dt.float32

    nc = bacc.Bacc("TRN2", target_bir_lowering=False, debug=False,
                   num_devices=NCORES)

    xT = nc.dram_tensor("xT", [2, 128, NTOK], bf, kind="ExternalInput")
    wq = nc.dram_tensor("wq", [128, 2, 2, 128], bf, kind="ExternalInput")
    wkv = nc.dram_tensor("wkv", [128, 2, 512], bf, kind="ExternalInput")
    thq = nc.dram_tensor("thq", [128, 2], f32, kind="ExternalInput")
    thkv = nc.dram_tensor("thkv", [128, 512], f32, kind="ExternalInput")
    wproj = nc.dram_tensor("wproj", [128, 2, 2, 128], bf, kind="ExternalInput")
    bproj = nc.dram_tensor("bproj", [128, 2], f32, kind="ExternalInput")
    bmask = nc.dram_tensor("bmask", [128, 129], bf, kind="ExternalInput")
    outT = nc.dram_tensor("outT", [2, 128, NTOK], f32, kind="ExternalOutput")
    sel_dbg = nc.dram_tensor("sel_dbg", [32, 32], f32, kind="ExternalOutput")
    reg_dbg = nc.dram_tensor("reg_dbg", [128, 64], f32, kind="ExternalOutput")

    cc_in = nc.dram_tensor("cc_in", [128, 64], f32)
    cc_out = nc.dram_tensor("cc_out", [128, 64], f32)

    with tile.TileContext(nc) as tc:
        with (
            tc.tile_pool(name="big", bufs=2) as big_pool,
            tc.tile_pool(name="persist", bufs=1) as pp,
            tc.tile_pool(name="kvs", bufs=6) as kv_pool,
            tc.tile_pool(name="gsb", bufs=3) as gsb_pool,
            tc.tile_pool(name="grow", bufs=4) as grow_pool,
            tc.tile_pool(name="small", bufs=2) as sm_pool,
            tc.tile_pool(name="attn_sb", bufs=4) as asb_pool,
            tc.tile_pool(name="outp", bufs=3) as out_pool,
            tc.tile_pool(name="mm512", bufs=4, space="PSUM") as mm512,
            tc.tile_pool(name="combp", bufs=2, space="PSUM") as combp,
            tc.tile_pool(name="attnp", bufs=2, space="PSUM") as attnp,
            tc.tile_pool(name="dram", bufs=1, space="DRAM") as dram_pool,
        ):
            # ---- load x (token-sliced so qkv can start early) ----
            xsb = big_pool.tile([128, 2, NTOK], bf, tag="bigbuf", bufs=1)
            for c in range(2):
                for p in range(4):
                    sl = slice(p * 2048, (p + 1) * 2048)
                    nc.sync.dma_start(xsb[:, c, sl], xT[c, :, sl])

            # ---- weights / constants ----
            wq_sb = pp.tile([128, 2, 2, 128], bf)
            nc.sync.dma_start(wq_sb[:], wq[:])
            wkv_sb = pp.tile([128, 2, 512], bf)
            nc.sync.dma_start(wkv_sb[:], wkv[:])
            thq_sb = pp.tile([128, 2], f32)
            nc.sync.dma_start(thq_sb[:], thq[:])
            thkv_sb = pp.tile([128, 512], f32)
            nc.sync.dma_start(thkv_sb[:], thkv[:])
            wproj_sb = pp.tile([128, 2, 2, 128], bf)
            nc.sync.dma_start(wproj_sb[:], wproj[:])
            bproj_sb = pp.tile([128, 2], f32)
            nc.sync.dma_start(bproj_sb[:], bproj[:])
            bmask_sb = pp.tile([128, 129], bf)
            nc.sync.dma_start(bmask_sb[:], bmask[:])
            id32 = pp.tile([32, 32], f32)
            make_identity(nc, id32[:])

            # ---- region partial sums -> collective (overlaps qkv) ----
            region = sm_pool.tile([128, 2, 32], f32, tag="region")
            for c in range(2):
                nc.vector.reduce_sum(
                    region[:, c, :],
                    xsb[:, c, :].rearrange("p (w s) -> p w s", s=S),
                    axis=mybir.AxisListType.X,
                )
            st = nc.sync.dma_start(cc_in[:], region[:].rearrange("p a w -> p (a w)"))
            cc = nc.gpsimd.collective_compute(
                "AllReduce", mybir.AluOpType.add, replica_groups=GROUPS,
                ins=[cc_in[:]], outs=[cc_out[:]],
            )
            add_dep_helper(cc.ins, st.ins, reason="region stored before collective")
            xs_sb = sm_pool.tile([128, 2, 32], f32, tag="xsum")
            ld = nc.sync.dma_start(xs_sb[:], cc_out[:].rearrange("p (a w) -> p a w", w=32))
            add_dep_helper(ld.ins, cc.ins, reason="collective before readback")
            nc.sync.dma_start(reg_dbg[:], cc_out[:])

            # ---- qkv + spikes + per-window Grams ----
            qsb = big_pool.tile([128, 2, NTOK], bf, tag="qsb", bufs=1)
            gram_dram = dram_pool.tile([32, 128, 2, 129], bf)
            kvts = {}
            for blk in range(16):
                tsl = slice(blk * 512, (blk + 1) * 512)
                for qc in range(2):
                    qp = mm512.tile([128, 512], f32, tag="mm512")
                    for c in range(2):
                        nc.tensor.matmul(qp[:], wq_sb[:, c, qc, :], xsb[:, c, tsl],
                                         start=(c == 0), stop=(c == 1))
                    nc.vector.tensor_scalar(qsb[:, qc, tsl], qp[:],
                                            thq_sb[:, qc:qc + 1], None,
                                            op0=mybir.AluOpType.is_ge)
                for tci in range(4):
                    tcg = blk * 4 + tci
                    ksl = slice(tcg * 128, (tcg + 1) * 128)
                    kvp = mm512.tile([128, 512], f32, tag="mm512")
                    for c in range(2):
                        nc.tensor.matmul(kvp[:], xsb[:, c, ksl], wkv_sb[:, c, :],
                                         start=(c == 0), stop=(c == 1))
                    kvt = kv_pool.tile([128, 513], bf, tag="kvt")
                    nc.any.memset(kvt[:, 512:513], 1.0)
                    nc.any.tensor_tensor(kvt[:, 0:512], kvp[:, 0:512],
                                         thkv_sb[:, 0:512], op=mybir.AluOpType.is_ge)
                    kvts[tcg] = kvt
                for w in (blk * 2, blk * 2 + 1):
                    t0, t1 = kvts[2 * w], kvts[2 * w + 1]
                    gsb = gsb_pool.tile([128, 2, 129], bf, tag="gsb")
                    for c in range(2):
                        gp = mm512.tile([128, 129], f32, tag="mm512")
                        rsl = slice(256 + c * 128, 256 + (c + 1) * 128)
                        ksl2 = slice(c * 128, (c + 1) * 128)
                        nc.tensor.matmul(gp[:, 0:128], t0[:, ksl2], t0[:, rsl],
                                         start=True, stop=False)
                        nc.tensor.matmul(gp[:, 0:128], t1[:, ksl2], t1[:, rsl],
                                         start=False, stop=True)
                        nc.tensor.matmul(gp[:, 128:129], t0[:, ksl2], t0[:, 512:513],
                                         start=True, stop=False)
                        nc.tensor.matmul(gp[:, 128:129], t1[:, ksl2], t1[:, 512:513],
                                         start=False, stop=True)
                        nc.vector.tensor_copy(gsb[:, c, :], gp[:])
                    nc.sync.dma_start(gram_dram[w], gsb[:])

            # ---- scores -> top-4 selection matrix sel / sel^T ----
            scp = combp.tile([32, 32], f32, tag="comb")
            for c in range(2):
                nc.tensor.matmul(scp[:], xs_sb[:, c, :], xs_sb[:, c, :],
                                 start=(c == 0), stop=(c == 1))
            shifted = sm_pool.tile([32, 32], f32, tag="shifted")
            nc.vector.tensor_scalar(shifted[:], scp[:], 1e6, None,
                                    op0=mybir.AluOpType.add)
            mx8 = sm_pool.tile([32, 8], f32, tag="mx8")
            nc.vector.max(mx8[:], shifted[:])
            nc.vector.memset(mx8[:, TOPK:], 0.0)
            zapped = sm_pool.tile([32, 32], f32, tag="zapped")
            nc.vector.match_replace(out=zapped[:], in_to_replace=mx8[:],
                                    in_values=shifted[:], imm_value=0.0)
            selb = sm_pool.tile([32, 32], f32, tag="selb")
            nc.vector.tensor_tensor(selb[:], shifted[:], zapped[:],
                                    op=mybir.AluOpType.is_gt)
            self_dbg = sm_pool.tile([32, 32], f32, tag="seldbg")
            nc.vector.tensor_copy(self_dbg[:], selb[:])
            nc.sync.dma_start(sel_dbg[:], self_dbg[:])
            selT_ps = combp.tile([32, 32], f32, tag="comb")
            nc.tensor.transpose(selT_ps[:], selb[:], id32[:])
            selT = sm_pool.tile([32, 32], bf, tag="selT")
            nc.vector.tensor_copy(selT[:], selT_ps[:])

            # ---- combine (sel^T @ gram rows), window-major via DRAM ----
            kvr_dram = dram_pool.tile([32, 128, 2, 129], bf)
            gflat = gram_dram[:].rearrange("w p c e -> w (p c e)")
            kflat = kvr_dram[:].rearrange("w p c e -> w (p c e)")
            kvread = asb_pool.tile([128, 2, 32, 129], bf, tag="kvread", bufs=1)
            dexp = asb_pool.tile([128, 2, 32, 128], bf, tag="dexp", bufs=1)
            selT4 = pp.tile([128, 32], bf)
            for j in range(4):
                nc.sync.dma_start(selT4[32 * j:32 * (j + 1), :], selT[:])
            for qtr in range(4):
                grow = grow_pool.tile([128, 6 * CCH], bf, tag="grow", bufs=2)
                for j in range(4):
                    jsl = slice(qtr * 24 * CCH + j * 6 * CCH,
                                qtr * 24 * CCH + (j + 1) * 6 * CCH)
                    nc.sync.dma_start(grow[32 * j:32 * (j + 1), :], gflat[:, jsl])
                kvout = grow_pool.tile([128, 6 * CCH], bf, tag="kvout", bufs=2)
                for ch in range(6):
                    csl = slice(ch * CCH, (ch + 1) * CCH)
                    cp = combp.tile([128, CCH], f32, tag="comb")
                    for j in range(4):
                        nc.tensor.matmul(cp[32 * j:32 * (j + 1), :],
                                         selT4[32 * j:32 * (j + 1), :],
                                         grow[32 * j:32 * (j + 1), csl],
                                         start=True, stop=True,
                                         tile_position=(32 * j, 32 * j))
                    nc.any.tensor_copy(kvout[:, csl], cp[:])
                for j in range(4):
                    jsl = slice(qtr * 24 * CCH + j * 6 * CCH,
                                qtr * 24 * CCH + (j + 1) * 6 * CCH)
                    nc.sync.dma_start(kflat[:, jsl], kvout[32 * j:32 * (j + 1), :])
            for c in range(2):
                # re-orient rows -> [hd, (w, e)]; den lhsT from the ksum column;
                # then mask to block-diagonal in place
                nc.sync.dma_start(
                    kvread[:, c, :, :],
                    kvr_dram[:, :, c, :].rearrange("w p e -> p w e"),
                )
                nc.vector.tensor_tensor(
                    dexp[:, c, :, :],
                    kvread[:, c, :, 128:129].to_broadcast([128, 32, 128]),
                    bmask_sb[:, None, 0:128].to_broadcast([128, 32, 128]),
                    op=mybir.AluOpType.mult,
                )
                nc.vector.tensor_tensor(
                    kvread[:, c, :, :], kvread[:, c, :, :],
                    bmask_sb[:, None, :].to_broadcast([128, 32, 129]),
                    op=mybir.AluOpType.mult,
                )

            # ---- attention + den + divide; then projection ----
            attn_nb = big_pool.tile([128, 2, NTOK], bf, tag="bigbuf", bufs=1)
            for blk in range(16):
                for w in (blk * 2, blk * 2 + 1):
                    wsl = slice(w * 256, (w + 1) * 256)
                    for c in range(2):
                        adp = attnp.tile([128, 512], f32, tag="attn")
                        nc.tensor.matmul(adp[:, 0:256], kvread[:, c, w, 0:128],
                                         qsb[:, c, wsl], start=True, stop=True)
                        nc.tensor.matmul(adp[:, 256:512], dexp[:, c, w, :],
                                         qsb[:, c, wsl], start=True, stop=True)
                        att_sc = out_pool.tile([128, 256], f32, tag="attsc", bufs=4)
                        nc.scalar.activation(att_sc[:], adp[:, 0:256],
                                             mybir.ActivationFunctionType.Copy)
                        den_sc = out_pool.tile([128, 256], f32, tag="densc", bufs=4)
                        nc.vector.tensor_scalar(den_sc[:], adp[:, 256:512],
                                                1e-6, None, op0=mybir.AluOpType.add)
                        nc.vector.reciprocal_approx_fast(
                            out=den_sc[:], in_=den_sc[:])
                        nc.vector.tensor_tensor(attn_nb[:, c, wsl], att_sc[:],
                                                den_sc[:], op=mybir.AluOpType.mult)
                tsl = slice(blk * 512, (blk + 1) * 512)
                for pc in range(2):
                    pjp = mm512.tile([128, 512], f32, tag="mm512")
                    for ec in range(2):
                        nc.tensor.matmul(pjp[:], wproj_sb[:, ec, pc, :],
                                         attn_nb[:, ec, tsl],
                                         start=(ec == 0), stop=(ec == 1))
                    osb = out_pool.tile([128, 512], f32, tag="osb")
                    nc.scalar.activation(osb[:], pjp[:],
                                         mybir.ActivationFunctionType.Identity,
                                         bias=bproj_sb[:, pc:pc + 1])
                    nc.sync.dma_start(outT[pc, :, tsl], osb[:])

    nc.compile()
    return nc


def _prep_shared(w_qkv, b_qkv, w_proj, b_proj):
    wq_a = w_qkv[:, 0:256].reshape(2, 128, 2, 128).transpose(1, 0, 2, 3)
    wkv_a = w_qkv[:, 256:768].reshape(2, 128, 512).transpose(1, 0, 2)
    th = 2.0 - b_qkv
    thq_a = th[0:256].reshape(2, 128).T
    thkv_a = np.broadcast_to(th[256:768], (128, 512))
    wproj_a = w_proj.reshape(2, 128, 2, 128).transpose(1, 0, 2, 3)
    bproj_a = b_proj.reshape(2, 128).T
    i = np.arange(128)[:, None]
    j = np.arange(129)[None, :]
    bmask_a = ((i // 32) == (j // 32)) | (j == 128)
    return {
        "wq": np.ascontiguousarray(wq_a).astype(BF16),
        "wkv": np.ascontiguousarray(wkv_a).astype(BF16),
        "thq": np.ascontiguousarray(thq_a).astype(np.float32),
        "thkv": np.ascontiguousarray(thkv_a).astype(np.float32),
        "wproj": np.ascontiguousarray(wproj_a).astype(BF16),
        "bproj": np.ascontiguousarray(bproj_a).astype(np.float32),
        "bmask": bmask_a.astype(BF16),
    }


def window_partition(x):
    """[T,B,Lt,Lh,Lw,C] -> [T,B,NTOK,C] with tokens in (w, s) order."""
    Tb, Bb = x.shape[0], x.shape[1]
    xw = x.reshape(Tb, Bb, WT, LT, WH, LH, WW, LW, C)
    xw = xw.transpose(0, 1, 2, 4, 6, 3, 5, 7, 8)
    return np.ascontiguousarray(xw).reshape(Tb, Bb, NTOK, C)


def window_reverse(o):
    """[NTOK, C] -> [Lt, Lh, Lw, C]."""
    o = o.reshape(WT, WH, WW, LT, LH, LW, C)
    o = o.transpose(0, 3, 1, 4, 2, 5, 6)
    return np.ascontiguousarray(o).reshape(Lt, Lh, Lw, C)


def run_kernel_spmd(nc, in_maps, **kwargs):
    from concourse.bass_utils import run_bass_kernel_spmd
    return run_bass_kernel_spmd(nc, in_maps, core_ids=list(range(NCORES)), **kwargs)


def kernel(x, w_qkv, b_qkv, w_proj, b_proj):
    x = np.asarray(x, dtype=np.float32)
    w_qkv = np.asarray(w_qkv, dtype=np.float32)
    b_qkv = np.asarray(b_qkv, dtype=np.float32)
    w_proj = np.asarray(w_proj, dtype=np.float32)
    b_proj = np.asarray(b_proj, dtype=np.float32)

    if "nc" not in _CACHE:
        _CACHE["nc"] = build_kernel()
    nc = _CACHE["nc"]

    shared = _prep_shared(w_qkv, b_qkv, w_proj, b_proj)
    xw = window_partition(x)
    in_maps = []
    for core in range(NCORES):
        b, t = core // 4, core % 4
        xt = np.ascontiguousarray(xw[t, b].T).astype(BF16)  # [C, NTOK]
        in_maps.append({**shared, "xT": xt.reshape(2, 128, NTOK)})

    res = run_kernel_spmd(nc, in_maps)

    out = np.empty((T, B, Lt, Lh, Lw, C), dtype=np.float32)
    for core in range(NCORES):
        b, t = core // 4, core % 4
        oT = res.results[core]["outT"].reshape(256, NTOK)
        out[t, b] = window_reverse(np.ascontiguousarray(oT.T))
    return out

